# revision 44
# baseline (speedup 1.0000x reference)
"""ConvDownsample2D (StyleGAN2 FIR blur + strided conv) for 8 Trainium2 cores.

Sharding: data-parallel over batch, 1 image per NeuronCore.

Per-core pipeline (all compute in fp16 with fp32 PSUM accumulation):
  1. x is passed in its NATIVE [H, W, C] layout as fp16 (no host transpose).
  2. The blur along H runs ON THE TENSOR ENGINE as a banded matmul
     y = x_htile.T @ B (contraction over image H), which also transposes
     NHWC into channel-major [C, w, h'] layout for free.
  3. The blur along W is a cascade of row-shifted adds on the vector engine
     ([1,3,3,1] = [1,1]^3), emitted in two row-windows per block so it
     overlaps the PE blur phase and unblocks early conv pairs.
  4. The 3x3/stride-2 conv is 9 accumulating matmuls per output column pair
     (lhsT = blurred activations [C,128pix], rhs = W taps [C,256oc]),
     INTERLEAVED into the PE stream with the banded-blur matmuls so the PE
     never idles waiting on PSUM evacuations. Bias is folded into the
     PSUM->SBUF evacuation (DVE tensor_tensor against a broadcast bias tile,
     or ACT copy + GpSimd bias add), casting to fp16.
  5. Output is stored as fp16 [OH, OW, OC] (per-partition-contiguous DMA)
     and upcast to fp32 on the host.
"""
import sys

if "/opt/trn_rl_repo" not in sys.path:
    sys.path.insert(0, "/opt/trn_rl_repo")

import numpy as np

import concourse.bass as bass
import concourse.tile as tile
from concourse import bacc, mybir
from concourse.bass_utils import run_bass_kernel_spmd

F16 = mybir.dt.float16
F32 = mybir.dt.float32

N_CORES = 8
H = W = 256
C = 128
OC = 256
OH = OW = 128
WP = W + 1          # 257 blurred extent
PITCH = 258         # even row pitch (fp16 4B alignment for DVE 2x mode)
XGRP = 12           # rows loaded per input DMA

import os as _os
_sizes = [int(v) for v in _os.environ.get(
    "KBLOCKS", "16,16,16,16,16,16,16,16").split(",")]
BLOCKS = []
_p = 0
for _s in _sizes:
    BLOCKS.append((_p, _s))
    _p += _s
assert _p == OH
N_YH = 2 * max(_sizes) + 4
N_YV = 2 * max(_sizes) + 1
WA = int(_os.environ.get("KWA", "17"))        # V-blur window-A extent
CV_START = int(_os.environ.get("KCVSTART", "3"))  # first hb pair with a cv after


def _build_bass(mode, repeat=1):
    nc = bacc.Bacc("TRN2", target_bir_lowering=False, debug=False)

    # native layout: first axis is the matmul-contracted (PE-blurred) dim
    x16 = nc.dram_tensor("x16", [H, W, C], F16, kind="ExternalInput").ap()
    b_ab = nc.dram_tensor("b_ab", [128, 261], F16, kind="ExternalInput").ap()
    w16 = nc.dram_tensor("w16", [9, C, OC], F16, kind="ExternalInput").ap()
    biasb = nc.dram_tensor("biasb", [128, 2, OC], F16, kind="ExternalInput").ap()
    if mode == "general":
        kvt = nc.dram_tensor("kvt", [128, 4], F32, kind="ExternalInput").ap()
    out = nc.dram_tensor("out", [OH, OW, OC], F16, kind="ExternalOutput").ap()

    if mode == "b1331":
        stage_shifts = [1, 1, 1]
    elif mode == "b1111":
        stage_shifts = [1, 2]
    else:
        stage_shifts = None

    with tile.TileContext(nc) as tc:
        with (
            tc.tile_pool(name="const", bufs=1) as cpool,
            tc.tile_pool(name="xin", bufs=10) as xpool,
            tc.tile_pool(name="yh", bufs=2) as yhpool,
            tc.tile_pool(name="tmp", bufs=2) as tmppool,
            tc.tile_pool(name="yv", bufs=2) as yvpool,
            tc.tile_pool(name="osb", bufs=4) as opool,
            tc.tile_pool(name="pyh", bufs=2, space=bass.MemorySpace.PSUM) as pyh,
            tc.tile_pool(name="pout", bufs=4, space=bass.MemorySpace.PSUM) as pout,
        ):
            bab_sb = cpool.tile([128, 261], F16)
            nc.sync.dma_start(bab_sb[:], b_ab[:])
            ba_sb = bab_sb[:, 0:131]
            bb_sb = bab_sb[:, 131:261]
            w_sb = cpool.tile([128, 9, OC], F16)
            biasb_sb = cpool.tile([128, 2, OC], F16)
            if mode == "general":
                kv_sb = cpool.tile([128, 4], F32, tag="kv_sb")
            else:
                kv_sb = None

            def load_weights():
                # deferred so block-0 input DMAs go first on the DGE queue
                for t in range(9):
                    nc.sync.dma_start(w_sb[:, t, :], w16[t])
                nc.sync.dma_start(biasb_sb[:], biasb[:])
                if mode == "general":
                    nc.sync.dma_start(kv_sb[:], kvt[:])

            def blur_setup(k):
                """Emit memsets + input DMAs for block k; return state for
                the interleaved emitters."""
                p0, pblk = BLOCKS[k]
                n_yh = 2 * pblk + 4
                hs0 = 2 * p0 - 2          # first y row (may be <0 / >=W)
                s_lo = max(0, -hs0)       # first valid slot
                s_hi = min(n_yh, W - hs0)  # end of valid slots (both even)
                # rows [0:4) of block k are rows [2*pblk_{k-1} : +4) of block
                # k-1: filled by an SBUF copy (halo dedup), not recomputed.
                base = s_lo if k == 0 else 4

                yh_t = yhpool.tile([128, N_YH, PITCH], F16, tag="yh")
                if s_lo > 0:
                    nc.gpsimd.memset(yh_t[:, 0:s_lo, :], 0.0)
                if s_hi < n_yh:
                    nc.gpsimd.memset(yh_t[:, s_hi:n_yh, :], 0.0)

                # fine-grained first loads for block 0 so the PE starts sooner
                sizes = []
                g0, left = base, s_hi - base
                if k == 0:
                    for s in (4, 6):
                        s = min(s, left)
                        if s:
                            sizes.append(s)
                            left -= s
                while left > 0:
                    s = min(XGRP, left)
                    sizes.append(s)
                    left -= s
                xtiles = []
                for sz in sizes:
                    pair = []
                    for t in range(2):
                        xt = xpool.tile([128, XGRP, C], F16, tag=f"x{t}")
                        nc.sync.dma_start(
                            xt[:, 0:sz, :],
                            x16[t * 128 : (t + 1) * 128, hs0 + g0 : hs0 + g0 + sz, :],
                        )
                        pair.append(xt)
                    xtiles.append((g0, sz, pair))
                    g0 += sz
                return (k, yh_t, base, s_hi, xtiles)

            def hb_pair(st, i):
                """Emit banded-blur matmuls + evac for 2-row pair i."""
                k, yh_t, s_lo, s_hi, xtiles = st
                s0 = s_lo + 2 * i

                def xslice(s):
                    for g0, xg, pair in xtiles:
                        if g0 <= s < g0 + xg:
                            return pair[0][:, s - g0, :], pair[1][:, s - g0, :]
                    raise AssertionError(s)

                pp = pyh.tile([128, 2, 512], F32)
                for e in range(2):
                    xlo, xhi = xslice(s0 + e)
                    nc.tensor.matmul(
                        pp[:, e, 0:131], xlo, ba_sb,
                        start=True, stop=True,
                    )
                    nc.tensor.matmul(
                        pp[:, e, 127:257], xhi, bb_sb,
                        start=False, stop=True, skip_group_check=True,
                    )
                # block 0's evac chain paces the first blur cascade: split it
                # across ACT and DVE there (DVE is idle during the fill)
                if k == 0 and i % 2 == 1:
                    nc.vector.tensor_copy(yh_t[:, s0 : s0 + 2, 0:WP], pp[:, :, 0:WP])
                else:
                    nc.scalar.copy(yh_t[:, s0 : s0 + 2, 0:WP], pp[:, :, 0:WP])

            def vblur_window(st, bufs, yv_t, r0, r1):
                """Emit the row-dim blur cascade producing yv rows [r0:r1)."""
                k, yh_t, s_lo, s_hi, xtiles = st
                if stage_shifts is not None:
                    nst = len(stage_shifts)
                    # Stage i output range: [a_i : r1 + tails_i). A non-zero
                    # r0 means an earlier window already produced stage-i
                    # rows below r0 + tails_i (and may have overwritten the
                    # inputs needed to recompute them) — start there.
                    tails = [sum(stage_shifts[i + 1:]) for i in range(nst)]
                    for i, sh in enumerate(stage_shifts):
                        last = i == nst - 1
                        src = bufs[i % 2]
                        dst = yv_t if last else bufs[(i + 1) % 2]
                        a = r0 + tails[i] if r0 > 0 else 0
                        b = r1 + tails[i]
                        nc.vector.tensor_add(
                            dst[:, a:b, 0:WP], src[:, a:b, 0:WP],
                            src[:, a + sh : b + sh, 0:WP],
                        )
                else:
                    nc.vector.tensor_scalar(
                        yv_t[:, r0:r1, 0:WP],
                        yh_t[:, r0:r1, 0:WP],
                        kv_sb[:, 0:1],
                        None,
                        mybir.AluOpType.mult,
                    )
                    for u in range(1, 4):
                        nc.vector.scalar_tensor_tensor(
                            yv_t[:, r0:r1, 0:WP],
                            yh_t[:, r0 + u : r1 + u, 0:WP],
                            kv_sb[:, u : u + 1],
                            yv_t[:, r0:r1, 0:WP],
                            mybir.AluOpType.mult,
                            mybir.AluOpType.add,
                        )

            def conv_pair(km1, yv_t, pr, evac_dve):
                """Emit conv 3x3 stride-2 for out column pair pr of block
                km1 + bias-fused evac + store."""
                p0, pblk = BLOCKS[km1]
                po = pout.tile([128, 2, OC], F32)  # one bank: 2 out cols
                for e in range(2):
                    r0 = 2 * (2 * pr + e)
                    for t in range(9):
                        a, b = divmod(t, 3)
                        lhsT = yv_t[:, r0 + a, b : b + 256 : 2]
                        nc.tensor.matmul(
                            po[:, e, :], lhsT, w_sb[:, t, :],
                            start=(e == 0 and t == 0),
                            stop=(t == 8),
                            skip_group_check=True,
                        )
                ot = opool.tile([128, 2, OC], F16)
                if evac_dve:
                    nc.vector.tensor_add(ot[:], po[:], biasb_sb[:])
                else:
                    nc.scalar.copy(ot[:], po[:])
                    nc.gpsimd.tensor_add(ot[:], ot[:], biasb_sb[:])
                p = p0 + 2 * pr
                nc.sync.dma_start(out[:, p : p + 2, :], ot[:])

            def pipe_block(k, prev, st, prefetch):
                """Emit blur of block k interleaved with conv of block k-1.
                Returns block k's yv tile."""
                p0, pblk = BLOCKS[k]
                n_yv = 2 * pblk + 1
                if k == 0 and not state["weights_loaded"]:
                    state["weights_loaded"] = True
                    load_weights()
                _, yh_t, s_lo, s_hi, _ = st
                nhb = (s_hi - s_lo) // 2
                yv_t = yvpool.tile([128, N_YV, PITCH], F16, tag="yv")
                tmp_t = tmppool.tile([128, N_YH, PITCH], F16, tag="tmp")
                bufs = [yh_t, tmp_t]

                ncv = BLOCKS[k - 1][1] // 2 if prev is not None else 0
                pyv = prev[0] if prev is not None else None

                # V-blur window split: each window fires once the hb pairs
                # covering its input rows are in. Block 0 gets a finer split
                # so the first conv pairs unblock as early as possible.
                ts = sum(stage_shifts) if stage_shifts is not None else 3
                if k == 0:
                    bounds = [0, 9, 17, n_yv]
                elif n_yv > WA + 3:
                    bounds = [0, WA, n_yv]
                else:
                    bounds = [0, n_yv]
                bounds = sorted(set(min(b, n_yv) for b in bounds))
                windows = []
                for r0, r1 in zip(bounds[:-1], bounds[1:]):
                    trig = min(nhb - 1, max(0, (r1 + ts + 1 - s_lo) // 2 - 1))
                    windows.append((trig, r0, r1))

                cv = 0          # next conv pair to emit
                hb_since = 0
                st_next = None

                def emit_cv(n):
                    nonlocal cv
                    for _ in range(n):
                        if cv < ncv:
                            conv_pair(k - 1, pyv, cv, evac_dve=(cv % 2 == 0))
                            cv += 1

                # halo dedup: this block's last 4 yh rows are the next
                # block's first 4. The copy must go out as soon as its
                # source pairs land — before any blur stage overwrites the
                # source rows (A stage-1 for single-window blocks, B
                # stage-1 for two-window ones).
                src_pair = (2 * pblk + 4 - s_lo) // 2 - 1

                def emit_halo_copy():
                    yh_next = st_next[1]
                    nc.vector.tensor_copy(
                        yh_next[:, 0:4, 0:WP],
                        yh_t[:, 2 * pblk : 2 * pblk + 4, 0:WP],
                    )

                if k == 1:
                    emit_cv(2)  # fill the PE while block 1's inputs land
                for i in range(nhb):
                    hb_pair(st, i)
                    hb_since += 1
                    if i == 0 and prefetch is not None:
                        st_next = prefetch()  # next block's input DMAs early
                    if i == src_pair and st_next is not None:
                        emit_halo_copy()
                    for trig, r0, r1 in windows:
                        if trig == i:
                            vblur_window(st, bufs, yv_t, r0, r1)
                    if hb_since >= 2 and i >= CV_START:
                        emit_cv(1)
                        hb_since = 0
                emit_cv(ncv)
                return (yv_t,)

            state = {"weights_loaded": False}
            for rep in range(repeat):
                prev = None
                sts = {0: blur_setup(0)}

                def make_prefetch(kn):
                    def pf():
                        if kn not in sts:
                            sts[kn] = blur_setup(kn)
                        return sts[kn]
                    return pf

                nblk = len(BLOCKS)
                for k in range(nblk):
                    if k not in sts:
                        sts[k] = blur_setup(k)
                    pf = make_prefetch(k + 1) if k + 1 < nblk else None
                    prev = pipe_block(k, prev, sts[k], pf)
                # drain: conv of the last block
                kl = nblk - 1
                for pr in range(BLOCKS[kl][1] // 2):
                    conv_pair(kl, prev[0], pr, evac_dve=True)
                sts.clear()

    nc.compile()
    return nc


_NC = {}


def _get_nc(mode="b1331", repeat=1):
    key = (mode, repeat)
    if key not in _NC:
        _NC[key] = _build_bass(mode, repeat)
    return _NC[key]


def _blur_mode(bk):
    k8 = bk / bk.sum() * 8.0
    if np.allclose(k8, [1.0, 3.0, 3.0, 1.0], rtol=1e-6, atol=1e-7):
        return "b1331"
    k4 = bk / bk.sum() * 4.0
    if np.allclose(k4, [1.0, 1.0, 1.0, 1.0], rtol=1e-6, atol=1e-7):
        return "b1111"
    return "general"


def _prepare_in_maps(x, conv_w, conv_b, blur_kernel):
    x = np.asarray(x, dtype=np.float32)
    conv_w = np.asarray(conv_w, dtype=np.float32)
    conv_b = np.asarray(conv_b, dtype=np.float32)
    bk = np.asarray(blur_kernel, dtype=np.float32)

    mode = _blur_mode(bk)
    k1 = (bk / bk.sum()).astype(np.float32)  # separable normalized taps

    # banded blur matrices for the PE-contracted dim (normalization folded in)
    Bfull = np.zeros((W, WP), np.float32)
    j = np.arange(W)[:, None]
    wp = np.arange(WP)[None, :]
    d = j - wp + 2
    m = (d >= 0) & (d <= 3)
    Bfull[m] = k1[d[m]]
    b_ab = np.concatenate(
        [Bfull[0:128, 0:131], Bfull[128:256, 127:257]], axis=1
    ).astype(np.float16)

    # row-dim normalization: box cascades compute the UNNORMALIZED sum,
    # so fold 1/sum(bk) into the conv weights for those modes.
    wscale = 1.0 / bk.sum() if mode in ("b1331", "b1111") else 1.0
    # The kernel's tile "rows" are the image W axis and its "columns" the
    # blurred H axis, so the 3x3 taps are transposed: row-tap = W tap.
    w16 = np.ascontiguousarray(
        (conv_w * wscale).transpose(1, 0, 2, 3).reshape(9, C, OC).astype(np.float16)
    )
    biasb = np.ascontiguousarray(
        np.broadcast_to(conv_b.astype(np.float16), (128, 2, OC))
    )

    in_maps = []
    for i in range(N_CORES):
        im = {
            "x16": np.ascontiguousarray(x[i].astype(np.float16)),
            "b_ab": b_ab,
            "w16": w16,
            "biasb": biasb,
        }
        if mode == "general":
            im["kvt"] = np.ascontiguousarray(
                np.broadcast_to(k1[None, :], (128, 4)).astype(np.float32)
            )
        in_maps.append(im)
    return mode, in_maps


def _run(mode, in_maps, **kwargs):
    nc = _get_nc(mode)
    return run_bass_kernel_spmd(nc, in_maps, core_ids=list(range(N_CORES)), **kwargs)


def kernel(x, conv_w, conv_b, blur_kernel):
    mode, in_maps = _prepare_in_maps(x, conv_w, conv_b, blur_kernel)
    res = _run(mode, in_maps)
    return np.stack(
        [res.results[i]["out"].astype(np.float32) for i in range(N_CORES)], axis=0
    )
